# revision 1
# baseline (speedup 1.0000x reference)
"""Trainium2 Bass kernel for nn_LunaCausalAttention.

Sharding: 8 cores; core c handles batch b = c//4 and heads hs = 4*(c%4) .. hs+4.
Each core computes its 4 heads' projections (feature-major, fp32r matmuls),
a chunked two-pass causal linear attention (C=128, head pairs packed into the
128-partition dim), and a partial output projection over its 256 head-features.
Host sums the 4 partials per batch (bias folded into rank-0's partial).
"""
import numpy as np

import concourse.bass as bass
import concourse.mybir as mybir
import concourse.tile as tile
from concourse import bacc
from concourse.masks import make_upper_triangular, make_identity
from concourse.bass_utils import run_bass_kernel_spmd

# static shapes
B, N, D, M, H, DH = 2, 1024, 1024, 64, 16, 64
C = 128                 # token chunk
NCH = N // C            # 8 chunks
NCORES = 8
HPC = 4                 # heads per core
E = HPC * DH            # 256 per-core head features
NF = D // 128           # 8 contraction tiles
BETA = float(np.log(2.0))
SCALE = DH ** -0.5

F32 = mybir.dt.float32
F32R = mybir.dt.float32r
BF16 = mybir.dt.bfloat16
ADT = BF16              # attention-core operand dtype
AF = mybir.ActivationFunctionType


def build_bass(phase=3):
    import os
    sub = int(os.environ.get("KSUB", "9"))
    nc = bacc.Bacc(None, target_bir_lowering=False)

    # ---- I/O ----
    xT_d = nc.dram_tensor("xT", [D, N], BF16, kind="ExternalInput")       # query[b].T
    pT_d = nc.dram_tensor("pT", [D, M], BF16, kind="ExternalInput")       # p[b].T
    wq_d = nc.dram_tensor("wq", [D, E], BF16, kind="ExternalInput")       # scale folded
    wk_d = nc.dram_tensor("wk", [D, E], BF16, kind="ExternalInput")
    wv_d = nc.dram_tensor("wv", [D, E], BF16, kind="ExternalInput")
    wpc_d = nc.dram_tensor("wpc", [D, E], BF16, kind="ExternalInput")
    wpq_d = nc.dram_tensor("wpq", [D, E], BF16, kind="ExternalInput")     # scale folded
    wo_d = nc.dram_tensor("wo", [E, D], BF16, kind="ExternalInput")
    bq_d = nc.dram_tensor("bq", [128, 2], F32, kind="ExternalInput")     # [i,et]=b[128et+i]
    bk_d = nc.dram_tensor("bk", [128, 2], F32, kind="ExternalInput")
    bpc_d = nc.dram_tensor("bpc", [128, 2], F32, kind="ExternalInput")
    bpq_d = nc.dram_tensor("bpq", [128, 2], F32, kind="ExternalInput")
    bkr_d = nc.dram_tensor("bkr", [1, E], BF16, kind="ExternalInput")     # row form
    bvr_d = nc.dram_tensor("bvr", [1, E], BF16, kind="ExternalInput")
    bor_d = nc.dram_tensor("bor", [1, D], BF16, kind="ExternalInput")     # bo or zeros
    rc_d = nc.dram_tensor("rc", [128, NCH], F32, kind="ExternalInput")   # 1/((i+1)*beta)
    ones_d = nc.dram_tensor("onesr", [1, 128], BF16, kind="ExternalInput")
    out_d = nc.dram_tensor("outp", [N, D], F32, kind="ExternalOutput")

    with tile.TileContext(nc) as tc:
        with (
            tc.tile_pool(name="singles", bufs=1) as singles,
            tc.tile_pool(name="work", bufs=4) as work,
            tc.tile_pool(name="obuf", bufs=3) as obuf,
            tc.tile_pool(name="psum", bufs=1, space="PSUM") as psum,
        ):
            # ---- constants ----
            triu = singles.tile([128, 2 * C], F32)      # two upper-tri copies
            make_upper_triangular(nc, triu[:, 0:C], val=1.0, diag=True)
            make_upper_triangular(nc, triu[:, C:2 * C], val=1.0, diag=True)
            ident = singles.tile([128, 128], F32)
            make_identity(nc, ident)
            identb = singles.tile([128, 128], ADT)
            make_identity(nc, identb)
            ones = singles.tile([1, 128], BF16)
            nc.sync.dma_start(out=ones, in_=ones_d[:, :])

            def load_w(name, dram):
                w = singles.tile([128, NF, E], BF16, name=name)
                nc.sync.dma_start(
                    out=w, in_=dram.rearrange("(f p) e -> p f e", p=128))
                return w

            # DMA in earliest-need order: pq projection inputs first, then
            # q/k/pc weights interleaved with xT tiles, then V/out weights.
            wpq_sb = load_w("wpq_sb", wpq_d)
            pT_sb = singles.tile([128, NF, M], BF16)
            nc.sync.dma_start(
                out=pT_sb, in_=pT_d.rearrange("(f p) m -> p f m", p=128))
            bpq_sb = singles.tile([128, 2], F32)
            nc.sync.dma_start(out=bpq_sb, in_=bpq_d[:, :])
            bq_sb = singles.tile([128, 2], F32)
            nc.sync.dma_start(out=bq_sb, in_=bq_d[:, :])
            bk_sb = singles.tile([128, 2], F32)
            nc.sync.dma_start(out=bk_sb, in_=bk_d[:, :])
            bpc_sb = singles.tile([128, 2], F32)
            nc.sync.dma_start(out=bpc_sb, in_=bpc_d[:, :])
            bvr_sb = singles.tile([1, E], BF16)
            nc.sync.dma_start(out=bvr_sb, in_=bvr_d[:, :])
            rc_sb = singles.tile([128, NCH], F32)
            nc.sync.dma_start(out=rc_sb, in_=rc_d[:, :])
            wq_sb = load_w("wq_sb", wq_d)
            wk_sb = load_w("wk_sb", wk_d)
            wpc_sb = load_w("wpc_sb", wpc_d)
            xt_sb = []
            for f in range(NF):
                xt = singles.tile([128, N], BF16, name=f"xt{f}")
                nc.sync.dma_start(out=xt, in_=xT_d[f * 128:(f + 1) * 128, :])
                xt_sb.append(xt)
            wv_sb = load_w("wv_sb", wv_d)
            wo_sb = singles.tile([128, 2, D], BF16)
            nc.sync.dma_start(
                out=wo_sb, in_=wo_d.rearrange("(t p) o -> p t o", p=128))
            bor_sb = singles.tile([1, D], BF16)
            nc.sync.dma_start(out=bor_sb, in_=bor_d[:, :])

            # projection outputs
            qT_sb = singles.tile([128, 2, N], ADT)    # [:, et, t] feature-major
            kT_sb = singles.tile([128, 2, N], ADT)
            pcT_sb = singles.tile([128, 2, N], ADT)
            pq_sb = singles.tile([128, 2, M], ADT)
            vtok_sb = [singles.tile([128, E], ADT, name=f"vtok{t}") for t in range(NCH)]
            attnT_sb = [singles.tile([128, 2, C], ADT, name=f"attnT{t}")
                        for t in range(NCH)]
            Sb_sb = singles.tile([128, 2, M], ADT)    # pass-1 state (bf16)
            Tb_sb = singles.tile([64, 4, DH], ADT)    # pass-2 state (bf16)
            nc.vector.memset(Sb_sb, 0.0)
            nc.vector.memset(Tb_sb, 0.0)

            # ---- pq projection: pqT (2 pair-tiles of (128, M)) ----
            for et in range(2):
                ppq = psum.tile([128, 512], F32, tag="pp", bufs=2, name="ppq")
                for f in range(NF):
                    nc.tensor.matmul(
                        ppq[:, 0:M], wpq_sb[:, f, et * 128:(et + 1) * 128],
                        pT_sb[:, f, :],
                        start=(f == 0), stop=(f == NF - 1))
                nc.vector.tensor_scalar_add(pq_sb[:, et, :], ppq[:, 0:M],
                                            bpq_sb[:, et:et + 1])

            # ---- feature-major projections: qT, kT, pcT ----
            def proj_et(et):
                for (dst, w, b) in ((qT_sb, wq_sb, bq_sb),
                                    (kT_sb, wk_sb, bk_sb),
                                    (pcT_sb, wpc_sb, bpc_sb)):
                    for nh in range(2):
                        pp = psum.tile([128, 512], F32, tag="pp", bufs=2, name="pp")
                        for f in range(NF):
                            nc.tensor.matmul(
                                pp, w[:, f, et * 128:(et + 1) * 128],
                                xt_sb[f][:, nh * 512:(nh + 1) * 512],
                                start=(f == 0), stop=(f == NF - 1))
                        nc.vector.tensor_scalar_add(
                            dst[:, et, nh * 512:(nh + 1) * 512], pp,
                            b[:, et:et + 1])

            proj_et(0)

            # ---- token-major projection: V_tok (K_tok comes from
            # per-chunk PE transposes of kT instead of a second projection)
            for tb in range(NCH):
                for (dst, w, brow) in ((vtok_sb, wv_sb, bvr_sb),):
                    pkv = psum.tile([128, 512], F32, tag="pp", bufs=2, name="pkv")
                    for f in range(NF):
                        nc.tensor.matmul(
                            pkv[:, 0:E], xt_sb[f][:, tb * 128:(tb + 1) * 128],
                            w[:, f, :],
                            start=(f == 0), stop=False)
                    nc.tensor.matmul(pkv[:, 0:E], ones, brow,
                                     start=False, stop=True)
                    nc.vector.tensor_copy(dst[tb], pkv[:, 0:E])

            if phase == 0:
                # debug: dump raw loaded tiles
                nc.sync.dma_start(out=out_d[0:128, :], in_=xt_sb[1][:, :].bitcast(F32))
                nc.sync.dma_start(out=out_d[128:256, 0:256],
                                  in_=wq_sb[:, 3, :].bitcast(F32))
                nc.sync.dma_start(out=out_d[256:384, 0:64],
                                  in_=pT_sb[:, 2, :].bitcast(F32))
                nc.sync.dma_start(out=out_d[384:512, 0:1024],
                                  in_=wo_sb[:, 1, :].bitcast(F32))

            if phase == 1:
                # debug: dump projections instead of attention
                for et in range(2):
                    nc.sync.dma_start(out=out_d[et * 128:(et + 1) * 128, :],
                                      in_=qT_sb[:, et, :])
                    nc.sync.dma_start(out=out_d[256 + et * 128:256 + (et + 1) * 128, :],
                                      in_=kT_sb[:, et, :])
                    nc.sync.dma_start(out=out_d[512 + et * 128:512 + (et + 1) * 128, :],
                                      in_=pcT_sb[:, et, :])
                for tb in range(4):
                    nc.sync.dma_start(
                        out=out_d[768 + tb * 64:768 + tb * 64 + 64, 0:256],
                        in_=ktok_sb[tb][0:64, :])

            # ---- attention chunks ----
            lo, hi = slice(0, 64), slice(64, 128)

            def attn_chunk(c, hp):
                tok = slice(c * C, (c + 1) * C)
                if True:
                    # Per-head banks: concurrent row-tiled (h0|h1) matmul
                    # pairs must drain into DIFFERENT psum banks (same-bank
                    # concurrent drains crash the PE).
                    pa = psum.tile([128, 256], F32, tag="pHA", bufs=2, name="pa")
                    pb = psum.tile([128, 256], F32, tag="pHB", bufs=2, name="pb")
                    pm = psum.tile([128, 512], F32, tag="pMD", name="pm")
                    ptr2 = pm[0:64, 256:384].bitcast(ADT).rearrange(
                        "p (h t) -> p h t", h=2)
                    pl = psum.tile([128, 384], F32, tag="pLT", name="pl")
                    pzs = (pa[:, 0:64], pb[:, 0:64])
                    paws = (pa[:, 64:128], pb[:, 64:128])
                    pgs = (pa[:, 128:256], pb[:, 128:256])
                    ppts = (ptr2[0:64, 0, :], ptr2[0:64, 1, :])
                    pan = pl[:, 0:128]
                    psd = pl[:, 128:192]
                    ptd = pl[0:64, 192:320]
                    pkt = pl[:, 320:384].bitcast(ADT)
                    sls = (lo, hi)
                    vh = (vtok_sb[c][:, hp * 128:hp * 128 + 64],
                          vtok_sb[c][:, hp * 128 + 64:hp * 128 + 128])
                    # K_tok for this chunk: one PE-transpose of the pair
                    ktc = work.tile([128, 128], ADT, name="ktc")
                    nc.tensor.transpose(pkt, kT_sb[:, hp, tok], identb)
                    nc.scalar.activation(ktc, pkt, AF.Copy)
                    kh = (ktc[:, 0:64], ktc[:, 64:128])

                    # pattn token-major (Z); AT derived by transposing Z
                    ez = work.tile([128, 128], F32, name="ez")
                    for h in (0, 1):
                        s = sls[h]
                        nc.tensor.matmul(pzs[h], pcT_sb[s, hp, tok],
                                         pq_sb[s, hp, :], start=True, stop=True,
                                         tile_position=(64 * h, 0))
                        nc.scalar.activation(ez[:, 64 * h:64 * h + 64], pzs[h],
                                             AF.Exp, scale=BETA)
                    z = work.tile([128, 128], ADT, name="z")
                    nc.scalar.activation(z, ez, AF.Ln, bias=1.0, scale=1.0)
                    patt = pm[0:64, 384:512].bitcast(ADT).rearrange(
                        "p (h t) -> p h t", h=2)
                    for h in (0, 1):
                        nc.tensor.transpose(patt[:, h, :],
                                            z[:, 64 * h:64 * h + 64], identb,
                                            tile_position=(0, 0))
                    at = work.tile([64, 2, 128], ADT, name="at")
                    nc.scalar.activation(at, patt, AF.Copy)

                    # ---- pass 1: G^T = K_c Q_c^T (j,i), masked ----
                    gm = work.tile([128, 256], ADT, name="gm")
                    for h in (0, 1):
                        s = sls[h]
                        nc.tensor.matmul(pgs[h], kT_sb[s, hp, tok],
                                         qT_sb[s, hp, tok], start=True,
                                         stop=True, tile_position=(64 * h, 0))
                        nc.vector.tensor_mul(gm[:, 128 * h:128 * h + 128],
                                             pgs[h], triu[:, 0:C])

                    # aw = Gm^T Z (+ Q S)
                    for h in (0, 1):
                        nc.tensor.matmul(paws[h], gm[:, 128 * h:128 * h + 128],
                                         z[:, 64 * h:64 * h + 64],
                                         start=True, stop=(c == 0))
                    if c > 0:
                        for h in (0, 1):
                            s = sls[h]
                            nc.tensor.matmul(paws[h], qT_sb[s, hp, tok],
                                             Sb_sb[s, hp, :], start=False,
                                             stop=True,
                                             tile_position=(64 * h, 0))

                    # ---- softmax (scales folded): P~ = exp(rc*aw)*rc/sum ----
                    ex = work.tile([128, 128], F32, name="ex")
                    rs = work.tile([128, 2], F32, name="rs")
                    for h in (0, 1):
                        nc.scalar.activation(ex[:, 64 * h:64 * h + 64], paws[h],
                                             AF.Exp, scale=rc_sb[:, c:c + 1],
                                             accum_out=rs[:, h:h + 1])
                    rcp = work.tile([128, 2], F32, name="rcp")
                    nc.vector.reciprocal(rcp, rs)
                    pt2 = work.tile([128, 128], ADT, name="pt2")
                    for h in (0, 1):
                        nc.vector.tensor_scalar(
                            pt2[:, 64 * h:64 * h + 64],
                            ex[:, 64 * h:64 * h + 64], rcp[:, h:h + 1],
                            rc_sb[:, c:c + 1], mybir.AluOpType.mult,
                            mybir.AluOpType.mult)

                    # transpose P~ -> (M, C) low tiles (psum partition 0)
                    for h in (0, 1):
                        nc.tensor.transpose(ppts[h], pt2[:, 64 * h:64 * h + 64],
                                            identb, tile_position=(0, 0))
                    ptT = work.tile([64, 256], ADT, name="ptT")
                    nc.scalar.activation(ptT, ptr2, AF.Copy)

                    # ---- pass 2: G2^T = Z_c P~^T via (AT, PT) ----
                    for h in (0, 1):
                        nc.tensor.matmul(pm[:, 128 * h:128 * h + 128],
                                         at[:, h, :],
                                         ptT[:, 128 * h:128 * h + 128],
                                         start=True, stop=True,
                                         tile_position=(0, 0))
                    g2m = work.tile([128, 256], ADT, name="g2m")
                    nc.vector.tensor_mul(g2m, pm[:, 0:256], triu)

                    # attn^T (pair tile (128, C)): V^T G2m (+ T^T P~^T)
                    for h in (0, 1):
                        nc.tensor.matmul(pan[64 * h:64 * h + 64, :], vh[h],
                                         g2m[:, 128 * h:128 * h + 128],
                                         start=True, stop=(c == 0),
                                         tile_position=(0, 64 * h))
                    if c > 0:
                        for h in (0, 1):
                            nc.tensor.matmul(pan[64 * h:64 * h + 64, :],
                                             Tb_sb[:, 2 * hp + h, :],
                                             ptT[:, 128 * h:128 * h + 128],
                                             start=False, stop=True,
                                             tile_position=(0, 64 * h))
                    nc.vector.tensor_copy(attnT_sb[c][:, hp, :], pan)

                    # ---- state updates ----
                    for h in (0, 1):
                        nc.tensor.matmul(psd[64 * h:64 * h + 64, :], kh[h],
                                         z[:, 64 * h:64 * h + 64],
                                         start=True, stop=True,
                                         tile_position=(0, 64 * h))
                        nc.tensor.matmul(ptd[:, 64 * h:64 * h + 64],
                                         z[:, 64 * h:64 * h + 64], vh[h],
                                         start=True, stop=True,
                                         tile_position=(0, 0))
                    nc.vector.tensor_add(Sb_sb[:, hp, :], Sb_sb[:, hp, :], psd)
                    nc.vector.tensor_add(
                        Tb_sb[:, 2 * hp:2 * hp + 2, :],
                        Tb_sb[:, 2 * hp:2 * hp + 2, :],
                        ptd.rearrange("p (h d) -> p h d", h=2))

            def out_block(c):
                tok = slice(c * C, (c + 1) * C)
                ob = obuf.tile([128, D], F32, name="ob")
                for oh in range(2):
                    po = psum.tile([128, 512], F32, tag="pp", bufs=2, name="po")
                    for et in range(2):
                        nc.tensor.matmul(
                            po, attnT_sb[c][:, et, :],
                            wo_sb[:, et, oh * 512:(oh + 1) * 512],
                            start=(et == 0), stop=False)
                    nc.tensor.matmul(po, ones,
                                     bor_sb[:, oh * 512:(oh + 1) * 512],
                                     start=False, stop=True)
                    nc.vector.tensor_copy(ob[:, oh * 512:(oh + 1) * 512], po)
                nc.sync.dma_start(out=out_d[tok, :], in_=ob)

            proj_et(1)
            if phase >= 2:
                for c in range(NCH):
                    attn_chunk(c, 0)
                    attn_chunk(c, 1)
                    out_block(c)

    # Patch the act-table map so Exp and Ln both resolve to the combined
    # natural_log_exp_and_others set (otherwise the load-placement pass
    # alternates exp_and_others <-> natural_log per chunk, ~42us of reloads).
    import concourse.bacc as _bacc_mod
    from concourse.hw_specs import get_activation_tables as _gat
    _orig_gat = _bacc_mod.get_activation_tables

    def _patched_gat(arch):
        t = _gat(arch)
        for name, s in t.items():
            if name != "natural_log_exp_and_others":
                s.discard(AF.Exp)
                s.discard(AF.Ln)
        return t

    _bacc_mod.get_activation_tables = _patched_gat
    try:
        nc.compile()
    finally:
        _bacc_mod.get_activation_tables = _orig_gat
    return nc


_CACHE = {}


import os


def _get_nc():
    phase = int(os.environ.get("KPHASE", "3"))
    key = f"nc{phase}_{os.environ.get('KSUB', '9')}"
    if key not in _CACHE:
        _CACHE[key] = build_bass(phase)
    return _CACHE[key]


def make_in_maps(query, p, Wq, bq, Wpq, bpq, Wpc, bpc, Wk, bk, Wv, bv, Wo, bo):
    import ml_dtypes
    bf = ml_dtypes.bfloat16
    f32 = lambda a: np.ascontiguousarray(np.asarray(a), dtype=np.float32)
    query, p = f32(query), f32(p)
    Wq, Wpq, Wpc, Wk, Wv, Wo = map(f32, (Wq, Wpq, Wpc, Wk, Wv, Wo))
    bq, bpq, bpc, bk, bv, bo = map(f32, (bq, bpq, bpc, bk, bv, bo))
    rc = (1.0 / ((np.arange(N) + 1.0) * BETA)).astype(np.float32)
    rc_cols = np.ascontiguousarray(rc.reshape(NCH, 128).T)

    def col2(v):  # (256,) -> (128, 2)
        return np.ascontiguousarray(v.reshape(2, 128).T)

    in_maps = []
    for core in range(NCORES):
        b = core // 4
        hs = (core % 4) * HPC
        cols = slice(hs * DH, (hs + HPC) * DH)
        m = {
            "xT": np.ascontiguousarray(query[b].T).astype(bf),
            "pT": np.ascontiguousarray(p[b].T).astype(bf),
            "wq": np.ascontiguousarray((Wq[cols, :] * SCALE).T).astype(bf),
            "wk": np.ascontiguousarray(Wk[cols, :].T).astype(bf),
            "wv": np.ascontiguousarray(Wv[cols, :].T).astype(bf),
            "wpc": np.ascontiguousarray(Wpc[cols, :].T).astype(bf),
            "wpq": np.ascontiguousarray((Wpq[cols, :] * SCALE).T).astype(bf),
            "wo": np.ascontiguousarray(Wo[:, cols].T).astype(bf),
            "bq": col2(bq[cols] * SCALE),
            "bk": col2(bk[cols]),
            "bpc": col2(bpc[cols]),
            "bpq": col2(bpq[cols] * SCALE),
            "bkr": np.ascontiguousarray(bk[cols].reshape(1, E)).astype(bf),
            "bvr": np.ascontiguousarray(bv[cols].reshape(1, E)).astype(bf),
            "bor": (bo.reshape(1, D).astype(bf) if core % 4 == 0
                    else np.zeros((1, D), bf)),
            "rc": rc_cols,
            "onesr": np.ones((1, 128), bf),
        }
        in_maps.append(m)
    return in_maps


def kernel(query, p, dec_input_mask=None, p_mask=None,
           Wq=None, bq=None, Wpq=None, bpq=None, Wpc=None, bpc=None,
           Wk=None, bk=None, Wv=None, bv=None, Wo=None, bo=None,
           _trace=False, _trace_kwargs=None):
    in_maps = make_in_maps(query, p, Wq, bq, Wpq, bpq, Wpc, bpc,
                           Wk, bk, Wv, bv, Wo, bo)
    res = run_bass_kernel_spmd(_get_nc(), in_maps, core_ids=list(range(NCORES)),
                               trace=_trace, **(_trace_kwargs or {}))
    out = np.zeros((B, N, D), np.float32)
    for core in range(NCORES):
        out[core // 4] += res.results[core]["outp"]
    if _trace:
        kernel.last_result = res
    return out



# revision 5
# speedup vs baseline: 1.2542x; 1.2542x over previous
"""Trainium2 Bass kernel for nn_LunaCausalAttention.

Sharding: 8 cores; core c handles batch b = c//4 and heads hs = 4*(c%4) .. hs+4.

Restructured vs baseline:
- DMA ordered so the pc projection streams against the xt tiles (no startup
  bubble); per-projection psum chains get enough banks to pipeline.
- Pass-1 computed m-major (awT = Z^T tril(G) + S^T Q), with rc folded into a
  pre-scaled qTrc at projection time, so the softmax exp emerges directly in
  the [m, tok] layout pass-2 needs -- no P~ transposes on the critical path.
- Softmax normalization deferred: P~ left unnormalized; a per-(head, token)
  scale tile (built by tiny fp16 outer-product matmuls) is applied once when
  attn^T leaves psum.
- S/T state accumulated directly in a persistent psum bank by the PE
  (start=c==0), copied to sbuf bf16 once per chunk on the Act engine.
- Output projection bias moved to the host-side partial reduction.
"""
import numpy as np

import concourse.bass as bass
import concourse.mybir as mybir
import concourse.tile as tile
from concourse import bacc
from concourse.masks import make_upper_triangular, make_identity
from concourse.bass_utils import run_bass_kernel_spmd

# static shapes
B, N, D, M, H, DH = 2, 1024, 1024, 64, 16, 64
C = 128                 # token chunk
NCH = N // C            # 8 chunks
NCORES = 8
HPC = 4                 # heads per core
E = HPC * DH            # 256 per-core head features
NF = D // 128           # 8 contraction tiles
BETA = float(np.log(2.0))
SCALE = DH ** -0.5

F32 = mybir.dt.float32
F16 = mybir.dt.float16
BF16 = mybir.dt.bfloat16
ADT = BF16              # attention-core operand dtype
AF = mybir.ActivationFunctionType
ALU = mybir.AluOpType


def build_bass():
    nc = bacc.Bacc(None, target_bir_lowering=False)

    # ---- I/O ----
    xT_d = nc.dram_tensor("xT", [D, N], BF16, kind="ExternalInput")       # query[b].T
    pT_d = nc.dram_tensor("pT", [D, M], BF16, kind="ExternalInput")       # p[b].T
    wq_d = nc.dram_tensor("wq", [D, E], BF16, kind="ExternalInput")       # scale folded
    wk_d = nc.dram_tensor("wk", [D, E], BF16, kind="ExternalInput")
    wv_d = nc.dram_tensor("wv", [D, E], BF16, kind="ExternalInput")
    wpc_d = nc.dram_tensor("wpc", [D, E], BF16, kind="ExternalInput")
    wpq_d = nc.dram_tensor("wpq", [D, E], BF16, kind="ExternalInput")     # scale folded
    wo_d = nc.dram_tensor("wo", [E, D], BF16, kind="ExternalInput")
    bq_d = nc.dram_tensor("bq", [128, 2], F32, kind="ExternalInput")      # [i,et]=b[128et+i]
    bk_d = nc.dram_tensor("bk", [128, 2], F32, kind="ExternalInput")
    bpc_d = nc.dram_tensor("bpc", [128, 2], F32, kind="ExternalInput")
    bpq_d = nc.dram_tensor("bpq", [128, 2], F32, kind="ExternalInput")
    bvr_d = nc.dram_tensor("bvr", [1, E], BF16, kind="ExternalInput")     # row form
    rcb_d = nc.dram_tensor("rcb", [128, N], F32, kind="ExternalInput")    # every row = rc
    ones_d = nc.dram_tensor("onesr", [1, 128], BF16, kind="ExternalInput")
    ones16_d = nc.dram_tensor("ones16", [1, 128], F16, kind="ExternalInput")
    ones2_d = nc.dram_tensor("ones2", [128, 2], BF16, kind="ExternalInput")  # h indicator
    out_d = nc.dram_tensor("outp", [N, D], F32, kind="ExternalOutput")

    with tile.TileContext(nc) as tc:
        with (
            tc.tile_pool(name="singles", bufs=1) as singles,
            tc.tile_pool(name="work", bufs=3) as work,
            tc.tile_pool(name="obuf", bufs=3) as obuf,
            tc.tile_pool(name="psum", bufs=1, space="PSUM") as psum,
        ):
            # ---- constants (device-generated) ----
            triu2 = singles.tile([128, 2 * C], F32)     # two upper-tri copies
            make_upper_triangular(nc, triu2[:, 0:C], val=1.0, diag=True)
            make_upper_triangular(nc, triu2[:, C:2 * C], val=1.0, diag=True)
            identb = singles.tile([128, 128], ADT)
            make_identity(nc, identb)

            # ---- DMA, in compute-need order ----
            def load_w(name, dram):
                w = singles.tile([128, NF, E], BF16, name=name)
                nc.sync.dma_start(
                    out=w, in_=dram.rearrange("(f p) e -> p f e", p=128))
                return w

            def load_small(shape, dt, dram, name):
                t = singles.tile(shape, dt, name=name)
                nc.sync.dma_start(out=t, in_=dram[:, :])
                return t

            wpc_sb = load_w("wpc_sb", wpc_d)
            bpc_sb = load_small([128, 2], F32, bpc_d, "bpc_sb")
            xt_sb = []
            for f in range(NF):
                xt = singles.tile([128, N], BF16, name=f"xt{f}")
                nc.sync.dma_start(out=xt, in_=xT_d[f * 128:(f + 1) * 128, :])
                xt_sb.append(xt)
            wk_sb = load_w("wk_sb", wk_d)
            bk_sb = load_small([128, 2], F32, bk_d, "bk_sb")
            wq_sb = load_w("wq_sb", wq_d)
            bq_sb = load_small([128, 2], F32, bq_d, "bq_sb")
            rcb_sb = singles.tile([128, N], F32)
            nc.sync.dma_start(out=rcb_sb, in_=rcb_d[:, :])
            wpq_sb = load_w("wpq_sb", wpq_d)
            pT_sb = singles.tile([128, NF, M], BF16)
            nc.sync.dma_start(
                out=pT_sb, in_=pT_d.rearrange("(f p) m -> p f m", p=128))
            bpq_sb = load_small([128, 2], F32, bpq_d, "bpq_sb")
            wv_sb = load_w("wv_sb", wv_d)
            bvr_sb = load_small([1, E], BF16, bvr_d, "bvr_sb")
            ones = load_small([1, 128], BF16, ones_d, "ones")
            ones16 = load_small([1, 128], F16, ones16_d, "ones16")
            ones2 = load_small([128, 2], BF16, ones2_d, "ones2")
            wo_sb = singles.tile([128, 2, D], BF16)
            nc.sync.dma_start(
                out=wo_sb, in_=wo_d.rearrange("(t p) o -> p t o", p=128))

            # ---- persistent sbuf tiles ----
            pcT_sb = singles.tile([128, 2, N], ADT)     # [feat, hp, tok]
            kT_sb = singles.tile([128, 2, N], ADT)
            qTrc_sb = singles.tile([128, 2, N], ADT)    # q * rc(tok), bias folded
            bdpq = singles.tile([128, 2, 128], ADT)     # block-diag pq per hp
            nc.vector.memset(bdpq, 0.0)
            vtok_sb = [singles.tile([128, E], ADT, name=f"vtok{t}")
                       for t in range(NCH)]
            attnT_sb = [singles.tile([128, 2, C], ADT, name=f"attnT{t}")
                        for t in range(NCH)]
            S_sb = [singles.tile([128, M], ADT, name=f"S{hp}") for hp in range(2)]
            Tb_sb = [singles.tile([64, 128], ADT, name=f"T{hp}") for hp in range(2)]

            # persistent psum state bank:
            #   S psum: [:, 0:64] hp0, [:, 64:128] hp1  (feat-pair x m)
            #   T psum: [0:64, 128:256] hp0, [0:64, 256:384] hp1  (m x feat-pair)
            #   rowsums: [64*hp : 64*hp+2, 384:512]
            state = psum.tile([128, 512], F32, tag="state", name="state")

            # ---- pc projection, f-streamed against xt DMA ----
            for et in range(2):
                for nh in range(2):
                    pp = psum.tile([128, 512], F32, tag="pp", bufs=2, name="ppc")
                    for f in range(NF):
                        nc.tensor.matmul(
                            pp, wpc_sb[:, f, et * 128:(et + 1) * 128],
                            xt_sb[f][:, nh * 512:(nh + 1) * 512],
                            start=(f == 0), stop=(f == NF - 1))
                    nc.scalar.activation(
                        pcT_sb[:, et, nh * 512:(nh + 1) * 512], pp,
                        AF.Identity, bias=bpc_sb[:, et:et + 1])

            # ---- k projection ----
            for et in range(2):
                for nh in range(2):
                    pp = psum.tile([128, 512], F32, tag="pp", bufs=2, name="ppk")
                    for f in range(NF):
                        nc.tensor.matmul(
                            pp, wk_sb[:, f, et * 128:(et + 1) * 128],
                            xt_sb[f][:, nh * 512:(nh + 1) * 512],
                            start=(f == 0), stop=(f == NF - 1))
                    nc.scalar.activation(
                        kT_sb[:, et, nh * 512:(nh + 1) * 512], pp,
                        AF.Identity, bias=bk_sb[:, et:et + 1])

            # ---- q projection -> qTrc = (q + bq) * rc ----
            for et in range(2):
                for nh in range(2):
                    pp = psum.tile([128, 512], F32, tag="pp", bufs=2, name="ppq")
                    for f in range(NF):
                        nc.tensor.matmul(
                            pp, wq_sb[:, f, et * 128:(et + 1) * 128],
                            xt_sb[f][:, nh * 512:(nh + 1) * 512],
                            start=(f == 0), stop=(f == NF - 1))
                    nc.vector.scalar_tensor_tensor(
                        qTrc_sb[:, et, nh * 512:(nh + 1) * 512], pp,
                        bq_sb[:, et:et + 1],
                        rcb_sb[:, nh * 512:(nh + 1) * 512],
                        ALU.add, ALU.mult)

            # ---- pq projection into block-diag layout ----
            for hp in range(2):
                ppq = psum.tile([128, 512], F32, tag="pp", bufs=2, name="pppq")
                for f in range(NF):
                    nc.tensor.matmul(
                        ppq[:, 0:M], wpq_sb[:, f, hp * 128:(hp + 1) * 128],
                        pT_sb[:, f, :],
                        start=(f == 0), stop=(f == NF - 1))
                for h in range(2):
                    s = slice(64 * h, 64 * h + 64)
                    nc.vector.tensor_scalar_add(
                        bdpq[s, hp, 64 * h:64 * h + 64], ppq[s, 0:M],
                        bpq_sb[s, hp:hp + 1])

            # ---- V projection (token-major, bias via ones-row matmul) ----
            for tb in range(NCH):
                pkv = psum.tile([128, 512], F32, tag="pp", bufs=2, name="pkv")
                for f in range(NF):
                    nc.tensor.matmul(
                        pkv[:, 0:E], xt_sb[f][:, tb * 128:(tb + 1) * 128],
                        wv_sb[:, f, :],
                        start=(f == 0), stop=False)
                nc.tensor.matmul(pkv[:, 0:E], ones, bvr_sb,
                                 start=False, stop=True)
                nc.vector.tensor_copy(vtok_sb[tb], pkv[:, 0:E])

            # ---- attention ----
            def attn_call(c, hp):
                tok = slice(c * C, (c + 1) * C)
                # psum packing
                A = psum.tile([128, 512], F32, tag="pca", bufs=2, name="A")
                Bp = psum.tile([128, 512], F32, tag="pcb", bufs=2, name="Bp")
                Cp = psum.tile([128, 512], F32, tag="pcc", bufs=1, name="Cp")
                pz = A[:, 0:128]
                awT = A[:, 128:256]
                gmp = (A[:, 256:384], Bp[:, 0:128])
                g2p = (A[:, 384:512], Bp[:, 128:256])
                pan = Bp[:, 256:384]
                scl = Bp[:, 384:512]
                pkt = Cp[:, 0:64].bitcast(ADT)
                att = Cp[:, 64:128].bitcast(ADT)
                rs = Cp[64 * hp:64 * hp + 2, 128:256]

                # Z_c: pz = pcT^T @ bdpq  -> [tok, m-pair]
                nc.tensor.matmul(pz, pcT_sb[:, hp, tok], bdpq[:, hp, :],
                                 start=True, stop=True)
                ez = work.tile([128, 128], F32, name="ez")
                nc.scalar.activation(ez, pz, AF.Exp, scale=BETA)
                z = work.tile([128, 128], ADT, name="z")
                nc.scalar.activation(z, ez, AF.Ln, bias=1.0, scale=1.0)

                # K_tok via PE transpose
                ktc = work.tile([128, 128], ADT, name="ktc")
                nc.tensor.transpose(pkt, kT_sb[:, hp, tok], identb)
                nc.vector.tensor_copy(ktc, pkt)

                # Z^T via PE transpose
                at = work.tile([128, 128], ADT, name="at")
                nc.tensor.transpose(att, z, identb)
                nc.scalar.activation(at, att, AF.Copy)

                # G^T = K Q_rc^T (rc folded in qTrc), masked
                gm = work.tile([128, 256], ADT, name="gm")
                for h in range(2):
                    s = slice(64 * h, 64 * h + 64)
                    nc.tensor.matmul(gmp[h], kT_sb[s, hp, tok],
                                     qTrc_sb[s, hp, tok], start=True, stop=True,
                                     tile_position=(64 * h, 0))
                    nc.vector.tensor_mul(gm[:, 128 * h:128 * h + 128],
                                         gmp[h], triu2[:, 0:C])

                # awT[m-pair, tok] = Z^T Gm (+ S^T Q_rc)
                for h in range(2):
                    s = slice(64 * h, 64 * h + 64)
                    nc.tensor.matmul(awT[s, :], z[:, s],
                                     gm[:, 128 * h:128 * h + 128],
                                     start=True, stop=(c == 0),
                                     tile_position=(0, 64 * h))
                if c > 0:
                    for h in range(2):
                        s = slice(64 * h, 64 * h + 64)
                        nc.tensor.matmul(awT[s, :], S_sb[hp][s, :],
                                         qTrc_sb[s, hp, tok],
                                         start=False, stop=True,
                                         tile_position=(0, 64 * h))

                # P~^T = exp(awT), unnormalized, directly m-major
                pt = work.tile([128, 128], ADT, name="pt")
                nc.scalar.activation(pt, awT, AF.Exp, scale=1.0)

                # rowsums over m (partition dim) via ones2 matmul -> [2, tok]
                nc.tensor.matmul(rs, ones2, pt, start=True, stop=True,
                                 tile_position=(0, 64 * hp))
                rcp = work.tile([2, 128], F32, name="rcp")
                nc.vector.reciprocal(rcp, rs)
                rcf = work.tile([2, 128], F16, name="rcf")
                nc.vector.tensor_mul(rcf, rcp, rcb_sb[0:2, tok])
                # scale tile: partitions 64h..64h+64 = rcf row h (broadcast)
                for h in range(2):
                    nc.tensor.matmul(scl[64 * h:64 * h + 64, :],
                                     ones16[:, 0:64], rcf[h:h + 1, :],
                                     start=True, stop=True,
                                     tile_position=(0, 64 * h))

                # pass 2: G2^T = Z P~^T, masked
                g2m = work.tile([128, 256], ADT, name="g2m")
                for h in range(2):
                    s = slice(64 * h, 64 * h + 64)
                    nc.tensor.matmul(g2p[h], at[s, :], pt[s, :],
                                     start=True, stop=True,
                                     tile_position=(0, 0))
                    nc.vector.tensor_mul(g2m[:, 128 * h:128 * h + 128],
                                         g2p[h], triu2[:, 0:C])

                # attn^T = V^T G2m (+ T^T P~^T), then normalize via scl
                for h in range(2):
                    nc.tensor.matmul(
                        pan[64 * h:64 * h + 64, :],
                        vtok_sb[c][:, hp * 128 + 64 * h:hp * 128 + 64 * h + 64],
                        g2m[:, 128 * h:128 * h + 128],
                        start=True, stop=(c == 0),
                        tile_position=(0, 64 * h))
                if c > 0:
                    for h in range(2):
                        s = slice(64 * h, 64 * h + 64)
                        nc.tensor.matmul(pan[s, :], Tb_sb[hp][:, s], pt[s, :],
                                         start=False, stop=True,
                                         tile_position=(0, 64 * h))
                nc.vector.tensor_mul(attnT_sb[c][:, hp, :], pan, scl)

                # ---- state updates (accumulate in psum, copy to sbuf) ----
                Sp = state[:, 64 * hp:64 * hp + 64]
                Tp = state[0:64, 128 + 128 * hp:256 + 128 * hp]
                for h in range(2):
                    s = slice(64 * h, 64 * h + 64)
                    nc.tensor.matmul(Sp[s, :], ktc[:, s], z[:, s],
                                     start=(c == 0), stop=True,
                                     tile_position=(0, 64 * h))
                    nc.tensor.matmul(
                        Tp[:, s], z[:, s],
                        vtok_sb[c][:, hp * 128 + 64 * h:hp * 128 + 64 * h + 64],
                        start=(c == 0), stop=True,
                        tile_position=(0, 0))
                if c < NCH - 1:
                    nc.scalar.copy(S_sb[hp], Sp)
                    nc.scalar.copy(Tb_sb[hp], Tp)

            def out_block(c):
                tok = slice(c * C, (c + 1) * C)
                for oh in range(2):
                    po = psum.tile([128, 512], F32, tag="pp", bufs=2, name="po")
                    for et in range(2):
                        nc.tensor.matmul(
                            po, attnT_sb[c][:, et, :],
                            wo_sb[:, et, oh * 512:(oh + 1) * 512],
                            start=(et == 0), stop=(et == 1))
                    ob = obuf.tile([128, 512], F32, name="ob")
                    if oh == 0:
                        nc.vector.tensor_copy(ob, po)
                    else:
                        nc.scalar.copy(ob, po)
                    nc.sync.dma_start(
                        out=out_d[tok, oh * 512:(oh + 1) * 512], in_=ob)

            for c in range(NCH):
                attn_call(c, 0)
                attn_call(c, 1)
                out_block(c)

    # Patch the act-table map so Exp and Ln both resolve to the combined
    # natural_log_exp_and_others set (otherwise the load-placement pass
    # alternates exp_and_others <-> natural_log per chunk, ~42us of reloads).
    import concourse.bacc as _bacc_mod
    from concourse.hw_specs import get_activation_tables as _gat
    _orig_gat = _bacc_mod.get_activation_tables

    def _patched_gat(arch):
        t = _gat(arch)
        for name, s in t.items():
            if name != "natural_log_exp_and_others":
                s.discard(AF.Exp)
                s.discard(AF.Ln)
        return t

    _bacc_mod.get_activation_tables = _patched_gat
    try:
        nc.compile()
    finally:
        _bacc_mod.get_activation_tables = _orig_gat
    return nc


_CACHE = {}


def _get_nc():
    if "nc" not in _CACHE:
        _CACHE["nc"] = build_bass()
    return _CACHE["nc"]


def make_in_maps(query, p, Wq, bq, Wpq, bpq, Wpc, bpc, Wk, bk, Wv, bv, Wo, bo):
    import ml_dtypes
    bf = ml_dtypes.bfloat16
    f32 = lambda a: np.ascontiguousarray(np.asarray(a), dtype=np.float32)
    query, p = f32(query), f32(p)
    Wq, Wpq, Wpc, Wk, Wv, Wo = map(f32, (Wq, Wpq, Wpc, Wk, Wv, Wo))
    bq, bpq, bpc, bk, bv, bo = map(f32, (bq, bpq, bpc, bk, bv, bo))
    rc = (1.0 / ((np.arange(N) + 1.0) * BETA)).astype(np.float32)
    rcb = np.ascontiguousarray(np.broadcast_to(rc[None, :], (128, N)))
    ones2 = np.zeros((128, 2), bf)
    ones2[0:64, 0] = 1
    ones2[64:128, 1] = 1

    def col2(v):  # (256,) -> (128, 2)
        return np.ascontiguousarray(v.reshape(2, 128).T)

    in_maps = []
    for core in range(NCORES):
        b = core // 4
        hs = (core % 4) * HPC
        cols = slice(hs * DH, (hs + HPC) * DH)
        m = {
            "xT": np.ascontiguousarray(query[b].T).astype(bf),
            "pT": np.ascontiguousarray(p[b].T).astype(bf),
            "wq": np.ascontiguousarray((Wq[cols, :] * SCALE).T).astype(bf),
            "wk": np.ascontiguousarray(Wk[cols, :].T).astype(bf),
            "wv": np.ascontiguousarray(Wv[cols, :].T).astype(bf),
            "wpc": np.ascontiguousarray(Wpc[cols, :].T).astype(bf),
            "wpq": np.ascontiguousarray((Wpq[cols, :] * SCALE).T).astype(bf),
            "wo": np.ascontiguousarray(Wo[:, cols].T).astype(bf),
            "bq": col2(bq[cols] * SCALE),
            "bk": col2(bk[cols]),
            "bpc": col2(bpc[cols]),
            "bpq": col2(bpq[cols] * SCALE),
            "bvr": np.ascontiguousarray(bv[cols].reshape(1, E)).astype(bf),
            "rcb": rcb,
            "onesr": np.ones((1, 128), bf),
            "ones16": np.ones((1, 128), np.float16),
            "ones2": ones2,
        }
        in_maps.append(m)
    return in_maps


def kernel(query, p, dec_input_mask=None, p_mask=None,
           Wq=None, bq=None, Wpq=None, bpq=None, Wpc=None, bpc=None,
           Wk=None, bk=None, Wv=None, bv=None, Wo=None, bo=None,
           _trace=False, _trace_kwargs=None):
    in_maps = make_in_maps(query, p, Wq, bq, Wpq, bpq, Wpc, bpc,
                           Wk, bk, Wv, bv, Wo, bo)
    res = run_bass_kernel_spmd(_get_nc(), in_maps, core_ids=list(range(NCORES)),
                               trace=_trace, **(_trace_kwargs or {}))
    bo = np.asarray(bo, dtype=np.float32)
    out = np.zeros((B, N, D), np.float32)
    out += bo.reshape(1, 1, D)
    for core in range(NCORES):
        out[core // 4] += res.results[core]["outp"]
    if _trace:
        kernel.last_result = res
    return out


# revision 6
# speedup vs baseline: 1.2549x; 1.0006x over previous
"""Trainium2 Bass kernel for nn_LunaCausalAttention.

Sharding: 8 cores; core c handles batch b = c//4 and heads hs = 4*(c%4) .. hs+4.

Restructured vs baseline:
- DMA ordered so the pc projection streams against the xt tiles (no startup
  bubble); per-projection psum chains get enough banks to pipeline.
- Pass-1 computed m-major (awT = Z^T tril(G) + S^T Q), with rc folded into a
  pre-scaled qTrc at projection time, so the softmax exp emerges directly in
  the [m, tok] layout pass-2 needs -- no P~ transposes on the critical path.
- Softmax normalization deferred: P~ left unnormalized; a per-(head, token)
  scale tile (built by tiny fp16 outer-product matmuls) is applied once when
  attn^T leaves psum.
- S/T state accumulated directly in a persistent psum bank by the PE
  (start=c==0), copied to sbuf bf16 once per chunk on the Act engine.
- Output projection bias moved to the host-side partial reduction.
"""
import numpy as np

import concourse.bass as bass
import concourse.mybir as mybir
import concourse.tile as tile
from concourse import bacc
from concourse.masks import make_upper_triangular, make_identity
from concourse.bass_utils import run_bass_kernel_spmd

# static shapes
B, N, D, M, H, DH = 2, 1024, 1024, 64, 16, 64
C = 128                 # token chunk
NCH = N // C            # 8 chunks
NCORES = 8
HPC = 4                 # heads per core
E = HPC * DH            # 256 per-core head features
NF = D // 128           # 8 contraction tiles
BETA = float(np.log(2.0))
SCALE = DH ** -0.5

F32 = mybir.dt.float32
F16 = mybir.dt.float16
BF16 = mybir.dt.bfloat16
ADT = BF16              # attention-core operand dtype
AF = mybir.ActivationFunctionType
ALU = mybir.AluOpType


def build_bass():
    nc = bacc.Bacc(None, target_bir_lowering=False)

    # ---- I/O ----
    xT_d = nc.dram_tensor("xT", [D, N], BF16, kind="ExternalInput")       # query[b].T
    pT_d = nc.dram_tensor("pT", [D, M], BF16, kind="ExternalInput")       # p[b].T
    wq_d = nc.dram_tensor("wq", [D, E], BF16, kind="ExternalInput")       # scale folded
    wk_d = nc.dram_tensor("wk", [D, E], BF16, kind="ExternalInput")
    wv_d = nc.dram_tensor("wv", [D, E], BF16, kind="ExternalInput")
    wpc_d = nc.dram_tensor("wpc", [D, E], BF16, kind="ExternalInput")
    wpq_d = nc.dram_tensor("wpq", [D, E], BF16, kind="ExternalInput")     # scale folded
    wo_d = nc.dram_tensor("wo", [E, D], BF16, kind="ExternalInput")
    bq_d = nc.dram_tensor("bq", [128, 2], F32, kind="ExternalInput")      # [i,et]=b[128et+i]
    bk_d = nc.dram_tensor("bk", [128, 2], F32, kind="ExternalInput")
    bpc_d = nc.dram_tensor("bpc", [128, 2], F32, kind="ExternalInput")
    bpq_d = nc.dram_tensor("bpq", [128, 2], F32, kind="ExternalInput")
    bvr_d = nc.dram_tensor("bvr", [1, E], BF16, kind="ExternalInput")     # row form
    rcb_d = nc.dram_tensor("rcb", [128, N], F32, kind="ExternalInput")    # every row = rc
    ones_d = nc.dram_tensor("onesr", [1, 128], BF16, kind="ExternalInput")
    o2T_d = nc.dram_tensor("o2T", [2, 128], F16, kind="ExternalInput")
    ones2_d = nc.dram_tensor("ones2", [128, 2], BF16, kind="ExternalInput")  # h indicator
    out_d = nc.dram_tensor("outp", [N, D], F32, kind="ExternalOutput")

    with tile.TileContext(nc) as tc:
        with (
            tc.tile_pool(name="singles", bufs=1) as singles,
            tc.tile_pool(name="work", bufs=3) as work,
            tc.tile_pool(name="obuf", bufs=3) as obuf,
            tc.tile_pool(name="psum", bufs=1, space="PSUM") as psum,
        ):
            # ---- constants (device-generated) ----
            triu2 = singles.tile([128, 2 * C], F32)     # two upper-tri copies
            make_upper_triangular(nc, triu2[:, 0:C], val=1.0, diag=True)
            make_upper_triangular(nc, triu2[:, C:2 * C], val=1.0, diag=True)
            identb = singles.tile([128, 128], ADT)
            make_identity(nc, identb)

            # ---- DMA, in compute-need order ----
            def load_w(name, dram):
                w = singles.tile([128, NF, E], BF16, name=name)
                nc.sync.dma_start(
                    out=w, in_=dram.rearrange("(f p) e -> p f e", p=128))
                return w

            def load_small(shape, dt, dram, name):
                t = singles.tile(shape, dt, name=name)
                nc.sync.dma_start(out=t, in_=dram[:, :])
                return t

            wpc_sb = load_w("wpc_sb", wpc_d)
            bpc_sb = load_small([128, 2], F32, bpc_d, "bpc_sb")
            xt_sb = []
            for f in range(NF):
                xt = singles.tile([128, N], BF16, name=f"xt{f}")
                nc.sync.dma_start(out=xt, in_=xT_d[f * 128:(f + 1) * 128, :])
                xt_sb.append(xt)
            wk_sb = load_w("wk_sb", wk_d)
            bk_sb = load_small([128, 2], F32, bk_d, "bk_sb")
            wq_sb = load_w("wq_sb", wq_d)
            bq_sb = load_small([128, 2], F32, bq_d, "bq_sb")
            rcb_sb = singles.tile([128, N], F32)
            nc.sync.dma_start(out=rcb_sb, in_=rcb_d[:, :])
            wpq_sb = load_w("wpq_sb", wpq_d)
            pT_sb = singles.tile([128, NF, M], BF16)
            nc.sync.dma_start(
                out=pT_sb, in_=pT_d.rearrange("(f p) m -> p f m", p=128))
            bpq_sb = load_small([128, 2], F32, bpq_d, "bpq_sb")
            wv_sb = load_w("wv_sb", wv_d)
            bvr_sb = load_small([1, E], BF16, bvr_d, "bvr_sb")
            ones = load_small([1, 128], BF16, ones_d, "ones")
            o2T = load_small([2, 128], F16, o2T_d, "o2T")
            ones2 = load_small([128, 2], BF16, ones2_d, "ones2")
            wo_sb = singles.tile([128, 2, D], BF16)
            nc.sync.dma_start(
                out=wo_sb, in_=wo_d.rearrange("(t p) o -> p t o", p=128))

            # ---- persistent sbuf tiles ----
            pcT_sb = singles.tile([128, 2, N], ADT)     # [feat, hp, tok]
            kT_sb = singles.tile([128, 2, N], ADT)
            qTrc_sb = singles.tile([128, 2, N], ADT)    # q * rc(tok), bias folded
            bdpq = singles.tile([128, 2, 128], ADT)     # block-diag pq per hp
            nc.vector.memset(bdpq, 0.0)
            vtok_sb = [singles.tile([128, E], ADT, name=f"vtok{t}")
                       for t in range(NCH)]
            attnT_sb = [singles.tile([128, 2, C], ADT, name=f"attnT{t}")
                        for t in range(NCH)]
            S_sb = [singles.tile([128, M], ADT, name=f"S{hp}") for hp in range(2)]
            Tb_sb = [singles.tile([64, 128], ADT, name=f"T{hp}") for hp in range(2)]

            # persistent psum state bank:
            #   S psum: [:, 0:64] hp0, [:, 64:128] hp1  (feat-pair x m)
            #   T psum: [0:64, 128:256] hp0, [0:64, 256:384] hp1  (m x feat-pair)
            #   rowsums: [64*hp : 64*hp+2, 384:512]
            state = psum.tile([128, 512], F32, tag="state", name="state")

            # ---- pc projection, f-streamed against xt DMA ----
            for et in range(2):
                for nh in range(2):
                    pp = psum.tile([128, 512], F32, tag="pp", bufs=2, name="ppc")
                    for f in range(NF):
                        nc.tensor.matmul(
                            pp, wpc_sb[:, f, et * 128:(et + 1) * 128],
                            xt_sb[f][:, nh * 512:(nh + 1) * 512],
                            start=(f == 0), stop=(f == NF - 1))
                    nc.scalar.activation(
                        pcT_sb[:, et, nh * 512:(nh + 1) * 512], pp,
                        AF.Identity, bias=bpc_sb[:, et:et + 1])

            # ---- k projection ----
            for et in range(2):
                for nh in range(2):
                    pp = psum.tile([128, 512], F32, tag="pp", bufs=2, name="ppk")
                    for f in range(NF):
                        nc.tensor.matmul(
                            pp, wk_sb[:, f, et * 128:(et + 1) * 128],
                            xt_sb[f][:, nh * 512:(nh + 1) * 512],
                            start=(f == 0), stop=(f == NF - 1))
                    nc.scalar.activation(
                        kT_sb[:, et, nh * 512:(nh + 1) * 512], pp,
                        AF.Identity, bias=bk_sb[:, et:et + 1])

            # ---- q projection -> qTrc = (q + bq) * rc ----
            for et in range(2):
                for nh in range(2):
                    pp = psum.tile([128, 512], F32, tag="pp", bufs=2, name="ppq")
                    for f in range(NF):
                        nc.tensor.matmul(
                            pp, wq_sb[:, f, et * 128:(et + 1) * 128],
                            xt_sb[f][:, nh * 512:(nh + 1) * 512],
                            start=(f == 0), stop=(f == NF - 1))
                    nc.vector.scalar_tensor_tensor(
                        qTrc_sb[:, et, nh * 512:(nh + 1) * 512], pp,
                        bq_sb[:, et:et + 1],
                        rcb_sb[:, nh * 512:(nh + 1) * 512],
                        ALU.add, ALU.mult)

            # ---- pq projection into block-diag layout ----
            for hp in range(2):
                ppq = psum.tile([128, 512], F32, tag="pp", bufs=2, name="pppq")
                for f in range(NF):
                    nc.tensor.matmul(
                        ppq[:, 0:M], wpq_sb[:, f, hp * 128:(hp + 1) * 128],
                        pT_sb[:, f, :],
                        start=(f == 0), stop=(f == NF - 1))
                for h in range(2):
                    s = slice(64 * h, 64 * h + 64)
                    nc.vector.tensor_scalar_add(
                        bdpq[s, hp, 64 * h:64 * h + 64], ppq[s, 0:M],
                        bpq_sb[s, hp:hp + 1])

            # ---- V projection (token-major, bias via ones-row matmul) ----
            for tb in range(NCH):
                pkv = psum.tile([128, 512], F32, tag="pp", bufs=2, name="pkv")
                for f in range(NF):
                    nc.tensor.matmul(
                        pkv[:, 0:E], xt_sb[f][:, tb * 128:(tb + 1) * 128],
                        wv_sb[:, f, :],
                        start=(f == 0), stop=False)
                nc.tensor.matmul(pkv[:, 0:E], ones, bvr_sb,
                                 start=False, stop=True)
                nc.vector.tensor_copy(vtok_sb[tb], pkv[:, 0:E])

            # ---- attention ----
            def attn_call(c, hp):
                tok = slice(c * C, (c + 1) * C)
                # psum packing
                A = psum.tile([128, 512], F32, tag="pca", bufs=2, name="A")
                Bp = psum.tile([128, 512], F32, tag="pcb", bufs=2, name="Bp")
                Cp = psum.tile([128, 512], F32, tag="pcc", bufs=1, name="Cp")
                pz = A[:, 0:128]
                awT = A[:, 128:256]
                gmp = (A[:, 256:384], Bp[:, 0:128])
                g2p = (A[:, 384:512], Bp[:, 128:256])
                pan = Bp[:, 256:384]
                scl = Bp[:, 384:512]
                pkt = Cp[:, 0:64].bitcast(ADT)
                att = Cp[:, 64:128].bitcast(ADT)
                rs = Cp[64 * hp:64 * hp + 2, 128:256]

                # Z_c: pz = pcT^T @ bdpq  -> [tok, m-pair]
                nc.tensor.matmul(pz, pcT_sb[:, hp, tok], bdpq[:, hp, :],
                                 start=True, stop=True)
                ez = work.tile([128, 128], F32, name="ez")
                nc.scalar.activation(ez, pz, AF.Exp, scale=BETA)
                z = work.tile([128, 128], ADT, name="z")
                nc.scalar.activation(z, ez, AF.Ln, bias=1.0, scale=1.0)

                # K_tok via PE transpose
                ktc = work.tile([128, 128], ADT, name="ktc")
                nc.tensor.transpose(pkt, kT_sb[:, hp, tok], identb)
                nc.vector.tensor_copy(ktc, pkt)

                # Z^T via PE transpose
                at = work.tile([128, 128], ADT, name="at")
                nc.tensor.transpose(att, z, identb)
                nc.scalar.activation(at, att, AF.Copy)

                # G^T = K Q_rc^T (rc folded in qTrc), masked
                gm = work.tile([128, 256], ADT, name="gm")
                for h in range(2):
                    s = slice(64 * h, 64 * h + 64)
                    nc.tensor.matmul(gmp[h], kT_sb[s, hp, tok],
                                     qTrc_sb[s, hp, tok], start=True, stop=True,
                                     tile_position=(64 * h, 0))
                    nc.vector.tensor_mul(gm[:, 128 * h:128 * h + 128],
                                         gmp[h], triu2[:, 0:C])

                # awT[m-pair, tok] = Z^T Gm (+ S^T Q_rc)
                for h in range(2):
                    s = slice(64 * h, 64 * h + 64)
                    nc.tensor.matmul(awT[s, :], z[:, s],
                                     gm[:, 128 * h:128 * h + 128],
                                     start=True, stop=(c == 0),
                                     tile_position=(0, 64 * h))
                if c > 0:
                    for h in range(2):
                        s = slice(64 * h, 64 * h + 64)
                        nc.tensor.matmul(awT[s, :], S_sb[hp][s, :],
                                         qTrc_sb[s, hp, tok],
                                         start=False, stop=True,
                                         tile_position=(0, 64 * h))

                # P~^T = exp(awT), unnormalized, directly m-major
                pt = work.tile([128, 128], ADT, name="pt")
                nc.scalar.activation(pt, awT, AF.Exp, scale=1.0)

                # rowsums over m (partition dim) via ones2 matmul -> [2, tok]
                nc.tensor.matmul(rs, ones2, pt, start=True, stop=True,
                                 tile_position=(0, 64 * hp))
                rcp = work.tile([2, 128], F32, name="rcp")
                nc.vector.reciprocal(rcp, rs)
                rcf = work.tile([2, 128], F16, name="rcf")
                nc.vector.tensor_mul(rcf, rcp, rcb_sb[0:2, tok])
                # scale tile [feat, tok]: partitions 64h..64h+64 = rcf row h
                nc.tensor.matmul(scl, o2T, rcf, start=True, stop=True)

                # pass 2: G2^T = Z P~^T, masked
                g2m = work.tile([128, 256], ADT, name="g2m")
                for h in range(2):
                    s = slice(64 * h, 64 * h + 64)
                    nc.tensor.matmul(g2p[h], at[s, :], pt[s, :],
                                     start=True, stop=True,
                                     tile_position=(0, 0))
                    nc.vector.tensor_mul(g2m[:, 128 * h:128 * h + 128],
                                         g2p[h], triu2[:, 0:C])

                # attn^T = V^T G2m (+ T^T P~^T), then normalize via scl
                for h in range(2):
                    nc.tensor.matmul(
                        pan[64 * h:64 * h + 64, :],
                        vtok_sb[c][:, hp * 128 + 64 * h:hp * 128 + 64 * h + 64],
                        g2m[:, 128 * h:128 * h + 128],
                        start=True, stop=(c == 0),
                        tile_position=(0, 64 * h))
                if c > 0:
                    for h in range(2):
                        s = slice(64 * h, 64 * h + 64)
                        nc.tensor.matmul(pan[s, :], Tb_sb[hp][:, s], pt[s, :],
                                         start=False, stop=True,
                                         tile_position=(0, 64 * h))
                nc.vector.tensor_mul(attnT_sb[c][:, hp, :], pan, scl)

                # ---- state updates (accumulate in psum, copy to sbuf) ----
                Sp = state[:, 64 * hp:64 * hp + 64]
                Tp = state[0:64, 128 + 128 * hp:256 + 128 * hp]
                for h in range(2):
                    s = slice(64 * h, 64 * h + 64)
                    nc.tensor.matmul(Sp[s, :], ktc[:, s], z[:, s],
                                     start=(c == 0), stop=True,
                                     tile_position=(0, 64 * h))
                    nc.tensor.matmul(
                        Tp[:, s], z[:, s],
                        vtok_sb[c][:, hp * 128 + 64 * h:hp * 128 + 64 * h + 64],
                        start=(c == 0), stop=True,
                        tile_position=(0, 0))
                if c < NCH - 1:
                    nc.scalar.copy(S_sb[hp], Sp)
                    nc.scalar.copy(Tb_sb[hp], Tp)

            def out_block(c):
                tok = slice(c * C, (c + 1) * C)
                for oh in range(2):
                    po = psum.tile([128, 512], F32, tag="pp", bufs=2, name="po")
                    for et in range(2):
                        nc.tensor.matmul(
                            po, attnT_sb[c][:, et, :],
                            wo_sb[:, et, oh * 512:(oh + 1) * 512],
                            start=(et == 0), stop=(et == 1))
                    ob = obuf.tile([128, 512], F32, name="ob")
                    if oh == 0:
                        nc.vector.tensor_copy(ob, po)
                    else:
                        nc.scalar.copy(ob, po)
                    nc.sync.dma_start(
                        out=out_d[tok, oh * 512:(oh + 1) * 512], in_=ob)

            for c in range(NCH):
                attn_call(c, 0)
                attn_call(c, 1)
                out_block(c)

    # Patch the act-table map so Exp and Ln both resolve to the combined
    # natural_log_exp_and_others set (otherwise the load-placement pass
    # alternates exp_and_others <-> natural_log per chunk, ~42us of reloads).
    import concourse.bacc as _bacc_mod
    from concourse.hw_specs import get_activation_tables as _gat
    _orig_gat = _bacc_mod.get_activation_tables

    def _patched_gat(arch):
        t = _gat(arch)
        for name, s in t.items():
            if name != "natural_log_exp_and_others":
                s.discard(AF.Exp)
                s.discard(AF.Ln)
        return t

    _bacc_mod.get_activation_tables = _patched_gat
    try:
        nc.compile()
    finally:
        _bacc_mod.get_activation_tables = _orig_gat
    return nc


_CACHE = {}


def _get_nc():
    if "nc" not in _CACHE:
        _CACHE["nc"] = build_bass()
    return _CACHE["nc"]


def make_in_maps(query, p, Wq, bq, Wpq, bpq, Wpc, bpc, Wk, bk, Wv, bv, Wo, bo):
    import ml_dtypes
    bf = ml_dtypes.bfloat16
    f32 = lambda a: np.ascontiguousarray(np.asarray(a), dtype=np.float32)
    query, p = f32(query), f32(p)
    Wq, Wpq, Wpc, Wk, Wv, Wo = map(f32, (Wq, Wpq, Wpc, Wk, Wv, Wo))
    bq, bpq, bpc, bk, bv, bo = map(f32, (bq, bpq, bpc, bk, bv, bo))
    rc = (1.0 / ((np.arange(N) + 1.0) * BETA)).astype(np.float32)
    rcb = np.ascontiguousarray(np.broadcast_to(rc[None, :], (128, N)))
    ones2 = np.zeros((128, 2), bf)
    ones2[0:64, 0] = 1
    ones2[64:128, 1] = 1
    o2T = np.zeros((2, 128), np.float16)
    o2T[0, 0:64] = 1
    o2T[1, 64:128] = 1

    def col2(v):  # (256,) -> (128, 2)
        return np.ascontiguousarray(v.reshape(2, 128).T)

    in_maps = []
    for core in range(NCORES):
        b = core // 4
        hs = (core % 4) * HPC
        cols = slice(hs * DH, (hs + HPC) * DH)
        m = {
            "xT": np.ascontiguousarray(query[b].T).astype(bf),
            "pT": np.ascontiguousarray(p[b].T).astype(bf),
            "wq": np.ascontiguousarray((Wq[cols, :] * SCALE).T).astype(bf),
            "wk": np.ascontiguousarray(Wk[cols, :].T).astype(bf),
            "wv": np.ascontiguousarray(Wv[cols, :].T).astype(bf),
            "wpc": np.ascontiguousarray(Wpc[cols, :].T).astype(bf),
            "wpq": np.ascontiguousarray((Wpq[cols, :] * SCALE).T).astype(bf),
            "wo": np.ascontiguousarray(Wo[:, cols].T).astype(bf),
            "bq": col2(bq[cols] * SCALE),
            "bk": col2(bk[cols]),
            "bpc": col2(bpc[cols]),
            "bpq": col2(bpq[cols] * SCALE),
            "bvr": np.ascontiguousarray(bv[cols].reshape(1, E)).astype(bf),
            "rcb": rcb,
            "onesr": np.ones((1, 128), bf),
            "o2T": o2T,
            "ones2": ones2,
        }
        in_maps.append(m)
    return in_maps


def kernel(query, p, dec_input_mask=None, p_mask=None,
           Wq=None, bq=None, Wpq=None, bpq=None, Wpc=None, bpc=None,
           Wk=None, bk=None, Wv=None, bv=None, Wo=None, bo=None,
           _trace=False, _trace_kwargs=None):
    in_maps = make_in_maps(query, p, Wq, bq, Wpq, bpq, Wpc, bpc,
                           Wk, bk, Wv, bv, Wo, bo)
    res = run_bass_kernel_spmd(_get_nc(), in_maps, core_ids=list(range(NCORES)),
                               trace=_trace, **(_trace_kwargs or {}))
    bo = np.asarray(bo, dtype=np.float32)
    out = np.zeros((B, N, D), np.float32)
    out += bo.reshape(1, 1, D)
    for core in range(NCORES):
        out[core // 4] += res.results[core]["outp"]
    if _trace:
        kernel.last_result = res
    return out
